# revision 42
# baseline (speedup 1.0000x reference)
"""Trainium2 Bass kernel for NeuronGemma4VisionAttention (v8).

Problem: B=2, P=4096, HID=1152, 16 heads x 72 dim, fp32 reference.
  q,k,v = x@Wq, x@Wk, x@Wv  -> per-head RMSNorm (q,k learned scale, v none)
  -> 2-part RoPE on q,k -> softmax(q k^T) v -> concat heads @ Wo

Sharding (8 cores, one chip):
  Head-parallel: core c owns heads (2c, 2c+1) for BOTH batches.
  Output-parallel: core c owns token slice [512c, 512c+512) of BOTH batches
  for the o_proj; one AllToAll per (batch, head-slot) redistributes
  head-slices to token-slices (rows padded 72->80 so the o_proj contraction
  stays 128-aligned via a host-padded Wo with 10 fi-chunks).

vs the 1342us baseline (measured 918-927us, rel err 5.4e-3):
  - fp16 operands throughout (x, Wqkv, rope coeffs, Q/K/identity/transposes,
    Wo, a2a payload); PSUM accumulation fp32; exp output / V stay float32r
    (exp spans e^+-60, fp16 would overflow). Q/K stay SBUF-resident.
  - Manual ACT table preload (exp_and_others) kills 257 alternating
    exp<->ln ACT_TABLE_LOADs (330us on the bottleneck ACT engine).
  - RMSNorm rsqrt on the DVE (exact reciprocal + linear seed + 2 Newton
    passes) instead of ACT Ln/Exp; softmax shift c_q = (B/2)(ss/t + t)
    >= B*|q| (AM-GM, t=12) needs no sqrt at all. Measured guard margins:
    rowmax-c <= 53 < 80 (no overflow), c-rowmax <= 68 < 85 (denominators
    stay normal f32); the c_q quantization error cancels exactly in
    softmax (per-query additive shift).
  - Softmax denominators ride row 96 of the PV psum (ones column in V);
    plain DVE reciprocal (reciprocal_approx_fast diverges on HW).
  - Emission is software-pipelined: scores(kb+2) ahead of exp(kb)/pv(kb);
    phase-1 of batch 1 is sliced into generator pieces woven between
    attention score groups of batch 0 so the exp stream never starves;
    o_proj(b0) pieces are woven into batch-1 attention likewise.
  - pso is copied to SBUF at chunk end so the psum bank frees early;
    normalization (reciprocal/broadcast/scale) runs off the critical path.
  - Per-(batch, head-slot) AllToAlls: the h0 collective fires halfway
    through each batch's attention and hides completely; the tail is one
    small collective + half an o_proj.
"""
import os
import sys

sys.path.insert(0, "/opt/trn_rl_repo")

import numpy as np

import concourse.bass as bass  # noqa: F401
import concourse.tile as tile
from concourse import bacc, mybir
from concourse.bass_utils import run_bass_kernel_spmd
from concourse.masks import make_identity

F32 = mybir.dt.float32
F32R = mybir.dt.float32r
F16 = mybir.dt.float16
AF = mybir.ActivationFunctionType

N_CORES = 8
B, P, HID = 2, 4096, 1152
NH, D = 16, 72
HL = 2                # heads per core
TB = B * P            # 8192 tokens across batches
NBLK = 64             # 128-token blocks total
KBLK = 32             # key blocks per batch
QC = 512              # query chunk
DP = D + 8            # padded per-head rows in the a2a payload
NQC = P // QC         # 8 query chunks per batch
BETA = 8.0
TNORM = 12.0          # AM-GM anchor ~ E[|rope(q_hat)|]
EPS = 1e-6
ACT_TABLE_EXP = 0     # exp_and_others in act_info.json

_CACHED_NC = None


def _build_nc():
    nc = bacc.Bacc("TRN2", target_bir_lowering=False, debug=False,
                   num_devices=N_CORES)

    xb = nc.dram_tensor("xb", [128, NBLK, 9, 128], F16,
                        kind="ExternalInput").ap()
    ropecb = nc.dram_tensor("ropecb", [128, NBLK, 4, D], F16,
                            kind="ExternalInput").ap()
    wqkv = nc.dram_tensor("wqkv", [128, 9, 3 * HL * D], F16,
                          kind="ExternalInput").ap()
    wo = nc.dram_tensor("wo", [128, 10, HID], F16,
                    kind="ExternalInput").ap()
    outT = nc.dram_tensor("outT", [HID, B, QC], F32,
                          kind="ExternalOutput").ap()
    dbg = {}
    if os.environ.get("KDBG"):
        dbg["kt00"] = nc.dram_tensor("dbg_kt00", [D + 1, P], F16,
                                     kind="ExternalOutput").ap()
        dbg["qt00"] = nc.dram_tensor("dbg_qt00", [D + 1, P], F16,
                                     kind="ExternalOutput").ap()
        dbg["va0"] = nc.dram_tensor("dbg_va0", [128, KBLK, HL, 97], F32R,
                                    kind="ExternalOutput").ap()
        dbg["ain0"] = nc.dram_tensor("dbg_ain0", [N_CORES, DP, QC], F16,
                                     kind="ExternalOutput").ap()
        dbg["sb0"] = nc.dram_tensor("dbg_sb0", [128, 3 * HL, D], F16,
                                    kind="ExternalOutput").ap()
        dbg["qaug0"] = nc.dram_tensor("dbg_qaug0", [128, HL, D + 1], F16,
                                      kind="ExternalOutput").ap()
        dbg["pso0"] = nc.dram_tensor("dbg_pso0", [97, QC], F32,
                                     kind="ExternalOutput").ap()
        dbg["rec0"] = nc.dram_tensor("dbg_rec0", [1, QC], F32,
                                     kind="ExternalOutput").ap()
        dbg["pt0"] = nc.dram_tensor("dbg_pt0", [128, QC], F32R,
                                    kind="ExternalOutput").ap()
        dbg["kaug0"] = nc.dram_tensor("dbg_kaug0", [128, HL, D + 1], F16,
                                      kind="ExternalOutput").ap()

    with tile.TileContext(nc) as tc:
        with (
            tc.tile_pool(name="persist", bufs=1) as persist,
            tc.tile_pool(name="dram", bufs=1, space="DRAM") as dram,
        ):
            # manual ACT table preload: one table holds Copy+Identity+Exp,
            # so the insert_act_table_loads fixpoint never needs another
            # load (and table 0's exp measured faster than table 6's).
            nc.scalar.add_instruction(mybir.InstLoadActFuncSet(
                name=nc.get_next_instruction_name(), ins=[], outs=[],
                act_func_set_id=ACT_TABLE_EXP))

            # ---- persistent state ----
            ident = persist.tile([128, 128], F16, tag="ident")
            make_identity(nc, ident)
            epst = persist.tile([128, 1], F32, tag="epst")
            nc.vector.memset(epst[:], EPS)
            cqb = persist.tile([128, 1], F32, tag="cqb")
            nc.vector.memset(cqb[:], -BETA * TNORM / 2.0)
            wqkv_sb = persist.tile([128, 9, 3 * HL * D], F16, tag="wqkv")
            nc.sync.dma_start(wqkv_sb[:], wqkv)
            wo_sb = persist.tile([128, 10, HID], F16, tag="wo")
            nc.sync.dma_start(wo_sb[:], wo)
            opart = persist.tile([128, 9, QC], F16, tag="opart")
            yt = [persist.tile([128, 5, QC], F16, tag=f"yt_{hl}",
                   name=f"yt_{hl}") for hl in range(HL)]
            kt = {}
            qt = {}
            for b in range(B):
                for hl in range(HL):
                    kt[(b, hl)] = persist.tile([D + 1, P], F16,
                                               tag=f"kt_{b}_{hl}",
                                               name=f"kt_{b}_{hl}")
                    qt[(b, hl)] = persist.tile([D + 1, P], F16,
                                               tag=f"qt_{b}_{hl}",
                                               name=f"qt_{b}_{hl}")
            # V padded to 97 cols: ones at col 96 (row 96 of the PV psum
            # holds the softmax denominators)
            vaug = [persist.tile([128, KBLK, HL, 97], F32R,
                                 tag=f"vaug_{b}", name=f"vaug_{b}")
                    for b in range(B)]
            for b in range(B):
                nc.vector.memset(vaug[b][:].bitcast(F32), 0.0)
                nc.vector.memset(vaug[b][:, :, :, 96].bitcast(F32), 1.0)

            a2a_in = [[dram.tile([N_CORES, DP, QC], F16,
                                 name=f"a2a_in_{b}_{h}",
                                 tag=f"a2a_in_{b}_{h}") for h in range(HL)]
                      for b in range(B)]
            a2a_out = [[dram.tile([N_CORES, DP, QC], F16,
                                  name=f"a2a_out_{b}_{h}",
                                  tag=f"a2a_out_{b}_{h}") for h in range(HL)]
                       for b in range(B)]

            # ---- pools ----  (entered so phase-1 pools pop first: LIFO)
            p2_cm = tc.tile_pool(name="p2", bufs=3)
            p2 = p2_cm.__enter__()
            p2sm_cm = tc.tile_pool(name="p2sm", bufs=2)
            p2sm = p2sm_cm.__enter__()
            p2s_cm = tc.tile_pool(name="p2s", bufs=3, space="PSUM")
            p2s = p2s_cm.__enter__()
            p2o_cm = tc.tile_pool(name="p2o", bufs=2, space="PSUM")
            p2o = p2o_cm.__enter__()
            p1d_cm = tc.tile_pool(name="p1d", bufs=4)
            p1d = p1d_cm.__enter__()
            p1_cm = tc.tile_pool(name="p1", bufs=2)
            p1 = p1_cm.__enter__()
            p1ps_cm = tc.tile_pool(name="p1ps", bufs=1, space="PSUM")
            p1ps = p1ps_cm.__enter__()
            trps_cm = tc.tile_pool(name="trps", bufs=2, space="PSUM")
            trps = trps_cm.__enter__()

            def p1_block_gen(blk, copies_on_act):
                b, kb = blk // KBLK, blk % KBLK
                cp = (lambda o, i: nc.scalar.activation(o, i, AF.Copy)) \
                    if copies_on_act else \
                    (lambda o, i: nc.vector.tensor_copy(o, i))

                xt = p1d.tile([128, 9, 128], F16, tag="xt")
                nc.sync.dma_start(xt[:], xb[:, blk])
                rc = p1d.tile([128, 4, D], F16, tag="rc")
                nc.sync.dma_start(rc[:], ropecb[:, blk])

                ps = p1ps.tile([128, 3 * HL * D], F32, tag="psqkv")
                for c in range(4):
                    nc.tensor.matmul(ps[:], xt[:, c, :], wqkv_sb[:, c, :],
                                     start=(c == 0), stop=False)
                yield
                for c in range(4, 9):
                    nc.tensor.matmul(ps[:], xt[:, c, :], wqkv_sb[:, c, :],
                                     start=False, stop=(c == 8))
                sb = p1.tile([128, 3 * HL, D], F16, tag="sb")
                cp(sb[:].rearrange("p g d -> p (g d)"), ps[:])
                yield

                # rms norm scales for the 6 (tensor, head) groups
                sq = p1.tile([128, 3 * HL, D], F16, tag="sq")
                nc.vector.tensor_mul(sq[:], sb[:], sb[:])
                ssr = p1.tile([128, 3 * HL], F32, tag="ssr")
                nc.vector.reduce_sum(ssr[:], sq[:],
                                     axis=mybir.AxisListType.X)
                # alpha = rsqrt(ssr/72 + eps) via DVE: linear seed in
                # u=1/m (fit over m in [0.12, 1.15]) + 2 Newton passes.
                AL = mybir.AluOpType
                m_t = p1.tile([128, 3 * HL], F32, tag="m_t")
                nc.vector.tensor_scalar(m_t[:], ssr[:], 1.0 / D, EPS,
                                        op0=AL.mult, op1=AL.add)
                u_t = p1.tile([128, 3 * HL], F32, tag="u_t")
                nc.vector.reciprocal(u_t[:], m_t[:])
                y_t = p1.tile([128, 3 * HL], F32, tag="y_t")
                nc.vector.tensor_scalar(y_t[:], u_t[:], 0.2670562903670214,
                                        0.8474368958486505,
                                        op0=AL.mult, op1=AL.add)
                t_t = p1.tile([128, 3 * HL], F32, tag="t_t")
                w_t = p1.tile([128, 3 * HL], F32, tag="w_t")
                y2_t = p1.tile([128, 3 * HL], F32, tag="y2_t")
                al = p1.tile([128, 3 * HL], F16, tag="al")
                nc.vector.tensor_mul(t_t[:], y_t[:], y_t[:])
                nc.vector.scalar_tensor_tensor(w_t[:], t_t[:], -0.5, m_t[:],
                                               op0=AL.mult, op1=AL.mult)
                nc.vector.scalar_tensor_tensor(y2_t[:], w_t[:], 1.5, y_t[:],
                                               op0=AL.add, op1=AL.mult)
                nc.vector.tensor_mul(t_t[:], y2_t[:], y2_t[:])
                nc.vector.scalar_tensor_tensor(w_t[:], t_t[:], -0.5, m_t[:],
                                               op0=AL.mult, op1=AL.mult)
                nc.vector.scalar_tensor_tensor(al[:], w_t[:], 1.5, y2_t[:],
                                               op0=AL.add, op1=AL.mult)
                yield

                # normalize: q,k -> qkh ; v -> vaug
                qkh = p1.tile([128, 2 * HL, D], F16, tag="qkh")
                nc.vector.tensor_mul(
                    qkh[:], sb[:, 0:2 * HL, :],
                    al[:, 0:2 * HL].unsqueeze(2).to_broadcast(
                        [128, 2 * HL, D]))
                nc.vector.tensor_mul(
                    vaug[b][:, kb, :, 0:D], sb[:, 2 * HL:3 * HL, :],
                    al[:, 2 * HL:3 * HL].unsqueeze(2).to_broadcast(
                        [128, HL, D]))

                # rope: out = qkh*cw + qkh_partner*sw (signs folded into sw)
                rc4 = rc[:].rearrange("p (r s) d -> p r s d", r=2)
                cw = rc4[:, :, 0, :]     # [128, 2(t), 72]
                sw5 = rc4[:, :, 1, :].rearrange(
                    "p r (a c j) -> p r a c j", a=2, c=2)
                rp = p1.tile([128, 2 * HL, D], F16, tag="rp")
                nc.vector.tensor_mul(
                    rp[:].rearrange("p (t h) d -> p t h d", t=2),
                    qkh[:].rearrange("p (t h) d -> p t h d", t=2),
                    cw.unsqueeze(2).to_broadcast([128, 2, HL, D]))
                yield
                rs = p1.tile([128, 2 * HL, 2, 2, 18], F16, tag="rs")
                qkh6 = qkh[:].rearrange("p g (a c j) -> p g a c j",
                                        a=2, c=2)
                for t in range(2):
                    gs = slice(t * HL, (t + 1) * HL)
                    for c in range(2):
                        swb = sw5[:, t, :, c, :].unsqueeze(1).to_broadcast(
                            [128, HL, 2, 18])
                        nc.vector.tensor_mul(rs[:, gs, :, c, :],
                                             qkh6[:, gs, :, 1 - c, :], swb)
                qaug = p1.tile([128, HL, D + 1], F16, tag="qaug")
                kaug = p1.tile([128, HL, D + 1], F16, tag="kaug")
                nc.vector.tensor_add(
                    qaug[:, :, 0:D], rp[:, 0:HL, :],
                    rs[:, 0:HL].rearrange("p g a c j -> p g (a c j)"))
                nc.vector.tensor_add(
                    kaug[:, :, 0:D], rp[:, HL:2 * HL, :],
                    rs[:, HL:2 * HL].rearrange("p g a c j -> p g (a c j)"))
                nc.vector.memset(kaug[:, :, D], 1.0)
                yield

                # c_q = (B/2)(|q|^2/t + t) >= B|q| ; col 72 of qaug = -c_q
                sqq = p1.tile([128, HL, D], F16, tag="sqq")
                nc.vector.tensor_mul(sqq[:], qaug[:, :, 0:D],
                                     qaug[:, :, 0:D])
                ss2 = p1.tile([128, HL], F32, tag="ss2")
                nc.vector.reduce_sum(ss2[:], sqq[:],
                                     axis=mybir.AxisListType.X)
                nc.scalar.activation(qaug[:, :, D], ss2[:], AF.Identity,
                                     scale=-BETA / (2.0 * TNORM),
                                     bias=cqb[:])

                if blk == 0 and dbg:
                    nc.sync.dma_start(dbg["sb0"], sb[:])
                    nc.sync.dma_start(dbg["qaug0"], qaug[:])
                    nc.sync.dma_start(dbg["kaug0"], kaug[:])
                # transpose q/k to feature-major SBUF
                ksl = slice(kb * 128, (kb + 1) * 128)
                for hl in range(HL):
                    yield
                    tq = trps.tile([D + 1, 128], F16, tag="tr", name="tq")
                    nc.tensor.transpose(tq[:], qaug[:, hl, :], ident[:])
                    cp(qt[(b, hl)][:, ksl], tq[:])
                    tk = trps.tile([D + 1, 128], F16, tag="tr", name="tk")
                    nc.tensor.transpose(tk[:], kaug[:, hl, :], ident[:])
                    cp(kt[(b, hl)][:, ksl], tk[:])

            def p2_chunk(b, hl, qc, feed=None):
                key = (b, hl)
                qsl = slice(qc * QC, (qc + 1) * QC)
                pso = p2o.tile([97, QC], F32, tag="pso")
                ps = {}
                pt = {}

                def scores(kb):
                    ps[kb] = p2s.tile([128, QC], F32, tag="ps2",
                                      name="ps2")
                    nc.tensor.matmul(ps[kb][:],
                                     kt[key][:, kb * 128:(kb + 1) * 128],
                                     qt[key][:, qsl],
                                     start=True, stop=True)

                scores(0)
                scores(1)
                for kb in range(KBLK):
                    if kb + 2 < KBLK:
                        scores(kb + 2)
                    pt[kb] = p2.tile([128, QC], F32R, tag="pt",
                                     name="pt")
                    nc.scalar.activation(pt[kb][:], ps[kb][:], AF.Exp)
                    nc.tensor.matmul(pso[:], vaug[b][:, kb, hl, :],
                                     pt[kb][:], start=(kb == 0),
                                     stop=(kb == KBLK - 1))
                    if feed is not None and kb % 2 == 1:
                        feed()

                oc = p2sm.tile([97, QC], F32, tag="oc")
                nc.vector.tensor_copy(oc[:], pso[:])
                rec = p2sm.tile([1, QC], F32, tag="rec")
                nc.vector.reciprocal(rec[:], oc[96:97, :])
                if dbg and b == 0 and hl == 0 and qc == 0:
                    nc.sync.dma_start(dbg["pso0"], oc[:])
                    nc.sync.dma_start(dbg["rec0"], rec[:])
                    nc.sync.dma_start(dbg["pt0"], pt[0][:])
                bct = p2sm.tile([D, QC], F32, tag="bct")
                nc.gpsimd.partition_broadcast(bct[:], rec[:])
                onrm = p2sm.tile([DP, QC], F16, tag="onrm")
                nc.vector.memset(onrm[64:DP, :], 0.0)
                nc.vector.tensor_mul(onrm[0:D, :], oc[0:D, :], bct[:])
                nc.sync.dma_start(a2a_in[b][hl][qc], onrm[:])

            def o_proj_p1_gen(b, p3, p3ps):
                # h0 half: runs right after the (b, h0) collective; partial
                # parked in fp16 SBUF
                nc.sync.dma_start(
                    yt[0][:],
                    a2a_out[b][0][:].rearrange(
                        "j r t -> (j r) t").rearrange(
                        "(c p) t -> p c t", p=128))
                for fo in range(9):
                    yield
                    ps3 = p3ps.tile([128, QC], F32, tag="ps3")
                    for fi in range(5):
                        nc.tensor.matmul(
                            ps3[:], wo_sb[:, fi, fo * 128:(fo + 1) * 128],
                            yt[0][:, fi, :], start=(fi == 0),
                            stop=(fi == 4))
                    nc.vector.tensor_copy(opart[:, fo, :], ps3[:])

            def o_proj_p2_gen(b, p3, p3ps):
                # h1 half + the parked partial re-injected via an identity
                # matmul into the psum accumulation
                nc.sync.dma_start(
                    yt[1][:],
                    a2a_out[b][1][:].rearrange(
                        "j r t -> (j r) t").rearrange(
                        "(c p) t -> p c t", p=128))
                for fo in range(9):
                    yield
                    ps3 = p3ps.tile([128, QC], F32, tag="ps3")
                    for fi in range(5, 10):
                        nc.tensor.matmul(
                            ps3[:], wo_sb[:, fi, fo * 128:(fo + 1) * 128],
                            yt[1][:, fi - 5, :], start=(fi == 5),
                            stop=False)
                    nc.tensor.matmul(ps3[:], ident[:], opart[:, fo, :],
                                     start=False, stop=True)
                    ot = p3.tile([128, QC], F32, tag="ot")
                    nc.vector.tensor_copy(ot[:], ps3[:])
                    nc.sync.dma_start(
                        outT[fo * 128:(fo + 1) * 128, b, :], ot[:])

            # ============ emission ============
            # phase 1, batch 0 (standalone: aux copies ride the idle ACT)
            for blk in range(KBLK):
                for _ in p1_block_gen(blk, copies_on_act=True):
                    pass

            # batch-0 attention with batch-1 phase-1 pieces woven between
            # score groups so the ACT exp stream never starves
            gen_q = [p1_block_gen(KBLK + i, copies_on_act=False)
                     for i in range(KBLK)]
            gen_q.reverse()

            def feed():
                while gen_q:
                    try:
                        next(gen_q[-1])
                        return
                    except StopIteration:
                        gen_q.pop()

            for j in range(2 * NQC):
                p2_chunk(0, j // NQC, j % NQC, feed=feed)
                feed()
                if j == NQC - 1:
                    nc.gpsimd.collective_compute(
                        "AllToAll", mybir.AluOpType.bypass,
                        ins=[a2a_in[0][0][:]], outs=[a2a_out[0][0][:]],
                        replica_groups=[list(range(N_CORES))],
                    )
            while gen_q:
                feed()
            if dbg:
                nc.sync.dma_start(dbg["kt00"], kt[(0, 0)][:])
                nc.sync.dma_start(dbg["qt00"], qt[(0, 0)][:])
                nc.sync.dma_start(dbg["va0"], vaug[0][:])
                nc.sync.dma_start(dbg["ain0"], a2a_in[0][0][:])
            trps_cm.__exit__(None, None, None)
            p1ps_cm.__exit__(None, None, None)
            p1_cm.__exit__(None, None, None)
            p1d_cm.__exit__(None, None, None)

            nc.gpsimd.collective_compute(
                "AllToAll", mybir.AluOpType.bypass,
                ins=[a2a_in[0][1][:]], outs=[a2a_out[0][1][:]],
                replica_groups=[list(range(N_CORES))],
            )
            p3_cm = tc.tile_pool(name="p3", bufs=2)
            p3 = p3_cm.__enter__()
            p3ps_cm = tc.tile_pool(name="p3ps", bufs=2, space="PSUM")
            p3ps = p3ps_cm.__enter__()

            # batch-1 attention; o_proj(b0) passes woven early (pass1 first
            # via LIFO), o_proj(b1) pass1 late after its h0 collective
            gen_q = [o_proj_p2_gen(0, p3, p3ps),
                     o_proj_p1_gen(0, p3, p3ps)]

            for j in range(2 * NQC):
                p2_chunk(1, j // NQC, j % NQC,
                         feed=feed if j >= 2 else None)
                if j == NQC - 1:
                    nc.gpsimd.collective_compute(
                        "AllToAll", mybir.AluOpType.bypass,
                        ins=[a2a_in[1][0][:]], outs=[a2a_out[1][0][:]],
                        replica_groups=[list(range(N_CORES))],
                    )
                if j == NQC + 1:
                    gen_q.append(o_proj_p1_gen(1, p3, p3ps))
            while gen_q:
                feed()

            nc.gpsimd.collective_compute(
                "AllToAll", mybir.AluOpType.bypass,
                ins=[a2a_in[1][1][:]], outs=[a2a_out[1][1][:]],
                replica_groups=[list(range(N_CORES))],
            )
            for _ in o_proj_p2_gen(1, p3, p3ps):
                pass

            p3ps_cm.__exit__(None, None, None)
            p3_cm.__exit__(None, None, None)
            p2o_cm.__exit__(None, None, None)
            p2s_cm.__exit__(None, None, None)
            p2sm_cm.__exit__(None, None, None)
            p2_cm.__exit__(None, None, None)

    nc.compile()
    return nc


def _prep_inputs(inputs):
    hs = np.asarray(inputs["hidden_states"], dtype=np.float32)
    cos = np.asarray(inputs["cos"], dtype=np.float32).reshape(TB, D)
    sin = np.asarray(inputs["sin"], dtype=np.float32).reshape(TB, D)
    Wq = np.asarray(inputs["Wq"], dtype=np.float32)
    Wk = np.asarray(inputs["Wk"], dtype=np.float32)
    Wv = np.asarray(inputs["Wv"], dtype=np.float32)
    Wo = np.asarray(inputs["Wo"], dtype=np.float32)
    qw = np.asarray(inputs["q_norm_w"], dtype=np.float32)
    kw = np.asarray(inputs["k_norm_w"], dtype=np.float32)

    # x: [HID, TB] -> blocked [p, blk, c, t]
    xT = hs.reshape(TB, HID).T.astype(np.float16)
    xb = np.ascontiguousarray(
        xT.reshape(9, 128, NBLK, 128).transpose(1, 2, 0, 3))

    # rope coeffs with norm weight and rotate-half sign folded:
    #   out[d] = q[d]*cw[d] + q[partner(d)]*sw[d]
    #   cw[d] = w[d]*cos[d]; sw[d] = sign(d)*sin[d]*w[partner(d)]
    partner = np.empty(D, np.int64)
    sign = np.empty(D, np.float32)
    for a in range(2):
        base = a * 36
        partner[base:base + 18] = np.arange(base + 18, base + 36)
        partner[base + 18:base + 36] = np.arange(base, base + 18)
        sign[base:base + 18] = -1.0
        sign[base + 18:base + 36] = 1.0
    ropec = np.stack([cos * qw[None, :],
                      sin * (sign * qw[partner])[None, :],
                      cos * kw[None, :],
                      sin * (sign * kw[partner])[None, :]],
                     axis=1).astype(np.float16)          # [TB, 4, 72]
    ropecb = np.ascontiguousarray(
        ropec.reshape(NBLK, 128, 4, D).transpose(1, 0, 2, 3))

    # pad Wo rows: layout (hl, core j, 72 real + 8 zero) = 1280 rows
    DPv = D + 8
    wo_pad = np.zeros((2 * N_CORES * DPv, HID), np.float32)
    for hl in range(2):
        for j in range(N_CORES):
            dst = (hl * N_CORES + j) * DPv
            srcr = (j * 2 + hl) * D
            wo_pad[dst:dst + D] = Wo[srcr:srcr + D]
    wob = np.ascontiguousarray(
        wo_pad.astype(np.float16).reshape(10, 128, HID).transpose(1, 0, 2))

    in_maps = []
    for c in range(N_CORES):
        colsl = slice(c * HL * D, (c + 1) * HL * D)
        wqkv = np.concatenate([Wq[:, colsl], Wk[:, colsl], Wv[:, colsl]],
                              axis=1).astype(np.float16)
        wqkvb = np.ascontiguousarray(
            wqkv.reshape(9, 128, 3 * HL * D).transpose(1, 0, 2))
        in_maps.append({
            "xb": xb,
            "ropecb": ropecb,
            "wqkv": wqkvb,
            "wo": wob,
        })
    return in_maps


def kernel(**inputs):
    global _CACHED_NC
    if _CACHED_NC is None:
        _CACHED_NC = _build_nc()
    nc = _CACHED_NC
    in_maps = _prep_inputs(inputs)
    trace = bool(int(os.environ.get("KERNEL_TRACE", "0")))
    res = run_bass_kernel_spmd(nc, in_maps, core_ids=list(range(N_CORES)),
                               trace=trace)
    kernel.last_results = res
    out = np.empty((B, P, HID), dtype=np.float32)
    for c in range(N_CORES):
        tsl = slice(c * QC, (c + 1) * QC)
        for b in range(B):
            out[b, tsl, :] = res.results[c]["outT"][:, b, :].T
    return out
